# revision 13
# baseline (speedup 1.0000x reference)
"""NTT (2^21-point, mod P=15*2^27+1) on 8 TRN2 NeuronCores.

Strategy: batch dim (8 rows) -> one row per core. Each row's NTT is a
radix-128^3 decomposition: three passes, each a 128-contraction matmul
done exactly on the PE via 8-bit limb decomposition in fp16 (products
<= 2^15, fp32 PSUM accumulation stays < 2^24 => bit-exact). Host does
the exact int64 twiddle multiplies / digit re-splits between passes.
"""

import numpy as np

P = 2013265921          # 15 * 2^27 + 1
GEN = 31
COSET = 7
LOG_N = 21
N = 1 << LOG_N          # 2097152 = 128^3
R = 128
F = N // R              # 16384 free columns per pass
BATCH = 8

_CACHE = {}


def _pow_table(base, length):
    t = np.ones(length, dtype=np.int64)
    k = 1
    while k < length:
        mult = pow(int(base), k, P)
        end = min(2 * k, length)
        t[k:end] = (t[:end - k] * mult) % P
        k = end
    return t


def _w128(omega):
    """Symmetric 128x128 matrix W[a,b] = omega^(2^14 * a * b) mod P."""
    w = pow(int(omega), 1 << 14, P)
    e = (np.arange(R, dtype=np.int64)[:, None] * np.arange(R, dtype=np.int64)[None, :])
    # exponents up to 127*127 < 2^14; table of w^j
    pt = _pow_table(w, R * R)
    return pt[e % (N)]  # e < 2^14 < N


def _balanced_digits(a):
    """a int64 in (-2^31, 2^31) -> 4 balanced base-256 digits in [-128,127]."""
    t = a.copy()
    digs = []
    for _ in range(4):
        d = ((t + 128) % 256) - 128
        digs.append(d)
        t = (t - d) >> 8
    assert np.all(t == 0), "balanced digit overflow"
    return digs


def _unsigned_digits(a):
    """a int64 in [0, 2^32) -> 4 unsigned base-256 digits."""
    return [(a >> (8 * j)) & 255 for j in range(4)]


def _build_pass_nc():
    import concourse.bass as bass
    import concourse.mybir as mybir
    from contextlib import ExitStack

    nc = bass.Bass("TRN2", target_bir_lowering=False)
    xd = nc.dram_tensor("xd", (R, 4, F), mybir.dt.float16, kind="ExternalInput")
    wd = nc.dram_tensor("wd", (R, 4, R), mybir.dt.float16, kind="ExternalInput")
    out = nc.dram_tensor("out", (R, 7, F), mybir.dt.float32, kind="ExternalOutput")

    TILE = 512              # one 2KB PSUM bank per s-plane
    NT = F // TILE
    with ExitStack() as ctx:
        sem_in = ctx.enter_context(nc.semaphore("sem_in"))
        sem_pe = ctx.enter_context(nc.semaphore("sem_pe"))
        sem_dve = ctx.enter_context(nc.semaphore("sem_dve"))
        sem_out = ctx.enter_context(nc.semaphore("sem_out"))
        wsb = ctx.enter_context(nc.sbuf_tensor("wsb", [R, 4, R], mybir.dt.float16))
        xsb = ctx.enter_context(nc.sbuf_tensor("xsb", [R, 4, F], mybir.dt.float16))
        ps = ctx.enter_context(nc.psum_tensor("ps", [R, 7, TILE], mybir.dt.float32))
        ob = ctx.enter_context(
            nc.sbuf_tensor("ob", [R, 2, 7, TILE], mybir.dt.float32))

        with nc.Block() as block:

            @block.gpsimd
            def _(g):
                g.dma_start(out=wsb[:], in_=wd[:, :, :]).then_inc(sem_in, 16)
                g.dma_start(out=xsb[:], in_=xd[:, :, :]).then_inc(sem_in, 16)

            @block.tensor
            def _(te):
                te.wait_ge(sem_in, 32)
                for k in range(NT):
                    if k >= 1:
                        te.wait_ge(sem_dve, k)      # psum drained by DVE
                    t0 = k * TILE
                    last = None
                    for st in range(7):
                        ijs = [(i, st - i)
                               for i in range(max(0, st - 3), min(3, st) + 1)]
                        for idx, (i, j) in enumerate(ijs):
                            last = te.matmul(
                                ps[:, st],
                                wsb[:, i],
                                xsb[:, j, t0:t0 + TILE],
                                start=(idx == 0),
                                stop=(idx == len(ijs) - 1),
                            )
                    last.then_inc(sem_pe)

            @block.vector
            def _(v):
                for k in range(NT):
                    v.wait_ge(sem_pe, k + 1)
                    if k >= 2:
                        v.wait_ge(sem_out, 16 * (k - 1))   # ob slot reusable
                    v.tensor_copy(ob[:, k % 2], ps[:]).then_inc(sem_dve)

            @block.sync
            def _(sy):
                for k in range(NT):
                    sy.wait_ge(sem_dve, k + 1)
                    sy.dma_start(out=out[:, :, k * TILE:(k + 1) * TILE],
                                 in_=ob[:, k % 2]).then_inc(sem_out, 16)
    return nc


def _get_nc():
    if "nc" not in _CACHE:
        _CACHE["nc"] = _build_pass_nc()
    return _CACHE["nc"]


_R_S = [pow(2, 8 * s, P) for s in range(7)]


def _run_pass(vals, wdig_f16, trace=False):
    """vals: int64 [8, 128, F] (contraction idx on axis 1, in [0,P)).
    Returns int64 [8, 128, F]: sum_p W[m,p]*vals[c,p,f] mod P per core c."""
    from concourse.bass_utils import run_bass_kernel_spmd

    nc = _get_nc()
    in_maps = []
    for c in range(BATCH):
        v = vals[c]
        xd = np.stack([d.astype(np.float16) for d in _unsigned_digits(v)],
                      axis=1)  # [128, 4, F]
        in_maps.append({"xd": xd, "wd": wdig_f16})
    import time as _time
    t0 = _time.time()
    try:
        res = run_bass_kernel_spmd(nc, in_maps, list(range(BATCH)), trace=trace)
    except Exception:
        if not trace:
            raise
        res = run_bass_kernel_spmd(nc, in_maps, list(range(BATCH)))
    _CACHE.setdefault("run_wall_s", []).append(_time.time() - t0)
    if trace and getattr(res, "exec_time_ns", None):
        _CACHE.setdefault("exec_ns", []).append(res.exec_time_ns)
    outs = np.stack([res.results[c]["out"] for c in range(BATCH)])  # [8,128,7,F]
    acc = np.zeros((BATCH, R, F), dtype=np.int64)
    for s in range(7):
        acc += outs[:, :, s, :].astype(np.int64) * _R_S[s]
    return acc % P


def _numpy_pass(vals, wdig):
    """Bit-exact numpy model of the device pass (for fallback/testing)."""
    acc = np.zeros((BATCH, R, vals.shape[2]), dtype=np.int64)
    wfull = sum(w * (1 << (8 * i)) for i, w in enumerate(wdig))  # == balanced W
    wT = wfull.T
    for c in range(BATCH):
        v_lo = vals[c] & 0xFFFF
        v_hi = vals[c] >> 16
        acc[c] = ((wT @ v_hi) % P * (1 << 16) + (wT @ v_lo)) % P
    return acc % P


def kernel(input, is_intt, is_coset, trace=False, use_device=True):
    x = np.asarray(input, dtype=np.int64) % P
    intt = bool(np.asarray(is_intt).item() if np.ndim(is_intt) == 0 else is_intt)
    coset = bool(np.asarray(is_coset).item() if np.ndim(is_coset) == 0 else is_coset)

    root = pow(GEN, (P - 1) // N, P)
    if intt:
        root = pow(root, P - 2, P)

    if (not intt) and coset:
        x = (x * _pow_table(COSET, N)[None, :]) % P

    # tables
    wt = _pow_table(root, N)                       # omega^e, e < N
    W = _w128(root)                                # symmetric 128x128
    Wb = W.copy()
    Wb[Wb > P // 2] -= P
    wdig = _balanced_digits(Wb)
    wdig_f16 = np.stack([d.astype(np.float16) for d in wdig], axis=1)  # [128,4,128]

    run = (lambda v: _run_pass(v, wdig_f16, trace)) if use_device else \
          (lambda v: _numpy_pass(v, wdig))

    # pass A: contract n1. x[c, n] with n = n1*2^14 + n2*2^7 + n3
    vA = x.reshape(BATCH, R, F)                    # [c, n1, (n2,n3)]
    yA = run(vA)                                   # [c, k1, (n2,n3)]

    # twiddle T1[k1, n2] = omega^(2^7 * n2 * k1)
    k1 = np.arange(R, dtype=np.int64)
    n2 = np.arange(R, dtype=np.int64)
    T1 = wt[((k1[:, None] * n2[None, :]) << 7) % N]        # [k1, n2]
    yA = yA.reshape(BATCH, R, R, R)                # [c, k1, n2, n3]
    yA = (yA * T1[None, :, :, None]) % P
    # pass B input: [c, n2, (k1, n3)]
    vB = np.ascontiguousarray(yA.transpose(0, 2, 1, 3)).reshape(BATCH, R, F)
    yB = run(vB)                                   # [c, k2, (k1, n3)]

    # twiddle T2[k2, k1, n3] = omega^(n3 * (2^7*k2 + k1))
    yB = yB.reshape(BATCH, R, R, R)                # [c, k2, k1, n3]
    n3 = np.arange(R, dtype=np.int64)
    e2 = (n3[None, None, :] * ((k1[:, None, None] << 7) + k1[None, :, None])) % N
    # e2 indexed [k2, k1, n3] (first axis k2 uses k1 array values 0..127)
    T2 = wt[e2]
    yB = (yB * T2[None, :, :, :]) % P
    # pass C input: [c, n3, (k1, k2)]
    vC = np.ascontiguousarray(yB.transpose(0, 3, 2, 1)).reshape(BATCH, R, F)
    yC = run(vC)                                   # [c, k3, (k1, k2)]

    yC = yC.reshape(BATCH, R, R, R)                # [c, k3, k1, k2]
    X = np.ascontiguousarray(yC.transpose(0, 1, 3, 2)).reshape(BATCH, N)
    # X array index = k3*2^14 + k2*2^7 + k1 ; value position k = k1 + 2^7 k2 + 2^14 k3  -> matches

    if intt:
        if coset:
            X = (X * _pow_table(pow(COSET, P - 2, P), N)[None, :]) % P
        n_inv = pow(N, P - 2, P)
        X = (X * n_inv) % P
    return X.astype(np.int64)
